# revision 3
# baseline (speedup 1.0000x reference)
"""Trainium2 Bass kernel for MindSpeed TE grouped linear (MoE grouped GEMM).

Computes, for E=64 experts with row splits m_splits (sum = 32768):
    y[rows_e, :] = x[rows_e, :] @ W[e].T        W[e]: [1408, 2048]

Strategy: pure expert-parallel over 8 NeuronCores — core c owns experts
[8c, 8c+8) and their (contiguous) token rows. No collectives; gather is a
host-side concat. Host pre-transposes both operands into K-major layout
([P=128 partitions = contraction chunk, KO=16, free]) so the PE consumes
both sides directly, and the matmuls run as float32r (FP22 truncate,
single-pass, full PE rate; ~1.5e-4 rel err).
"""

import math

import numpy as np

import concourse.mybir as mybir
import concourse.tile as tile
from concourse import bacc
from concourse.bass_utils import run_bass_kernel_spmd

N_CORES = 8
P = 128
IN_SIZE = 2048
OUT_SIZE = 1408
KO = IN_SIZE // P  # 16 contraction subtiles
N_TILE = 512

_nc_cache: dict = {}


def _n_tiles():
    tiles = []
    n0 = 0
    while n0 < OUT_SIZE:
        nsz = min(N_TILE, OUT_SIZE - n0)
        tiles.append((n0, nsz))
        n0 += nsz
    return tiles


def _build(pattern: tuple) -> "bacc.Bacc":
    """One SPMD program: `pattern` = per-expert (padded) token counts for the
    8 local experts of a core; identical across cores."""
    T = sum(pattern)
    m_max = max(pattern)
    E_loc = len(pattern)
    nc = bacc.Bacc(None, target_bir_lowering=False, name="grouped_linear")
    xT = nc.dram_tensor("xT", [P, KO, T], mybir.dt.float32r, kind="ExternalInput")
    wT = nc.dram_tensor(
        "wT", [E_loc, P, KO, OUT_SIZE], mybir.dt.float32r, kind="ExternalInput"
    )
    y = nc.dram_tensor("y", [T, OUT_SIZE], mybir.dt.float32, kind="ExternalOutput")

    KH = KO // 2  # W arrives in two half-K chunks for finer pipelining
    with tile.TileContext(nc) as tc:
        with (
            tc.tile_pool(name="xp", bufs=8) as xpool,
            tc.tile_pool(name="wp", bufs=6) as wpool,
            tc.tile_pool(name="op", bufs=4) as opool,
            tc.tile_pool(name="ps", bufs=6, space="PSUM") as pspool,
        ):
            t0 = 0
            for e in range(E_loc):
                m = pattern[e]
                if m == 0:
                    continue
                mts = m // P
                x_ts = []
                for mt in range(mts):
                    x_t = xpool.tile([P, KO, P], mybir.dt.float32r, tag="x", name="x_t")
                    nc.sync.dma_start(
                        x_t[:], xT[:, :, t0 + mt * P : t0 + (mt + 1) * P]
                    )
                    x_ts.append(x_t)
                for n0, nsz in _n_tiles():
                    w_hs = []
                    for kh in range(2):
                        w_h = wpool.tile(
                            [P, KH, N_TILE], mybir.dt.float32r, tag="w", name="w_h"
                        )
                        nc.sync.dma_start(
                            w_h[:, :, :nsz],
                            wT[e, :, kh * KH : (kh + 1) * KH, n0 : n0 + nsz],
                        )
                        w_hs.append(w_h)
                    for mt in range(mts):
                        ps_t = pspool.tile(
                            [P, N_TILE], mybir.dt.float32, tag="ps", name="ps_t"
                        )
                        for kh in range(2):
                            for k in range(KH):
                                ko = kh * KH + k
                                nc.tensor.matmul(
                                    ps_t[:, :nsz],
                                    x_ts[mt][:, ko, :],
                                    w_hs[kh][:, k, :nsz],
                                    start=(ko == 0),
                                    stop=(ko == KO - 1),
                                )
                        o_t = opool.tile(
                            [P, N_TILE], mybir.dt.float32, tag="o", name="o_t"
                        )
                        nc.vector.tensor_copy(o_t[:, :nsz], ps_t[:, :nsz])
                        nc.sync.dma_start(
                            y[t0 + mt * P : t0 + (mt + 1) * P, n0 : n0 + nsz],
                            o_t[:, :nsz],
                        )
                t0 += m
    nc.compile()
    return nc


def _get_nc(pattern: tuple) -> "bacc.Bacc":
    nc = _nc_cache.get(pattern)
    if nc is None:
        nc = _build(pattern)
        _nc_cache[pattern] = nc
    return nc


def _plan(splits: np.ndarray):
    """Choose a per-core expert-size pattern (identical across cores, sizes
    multiples of 128). Returns (padded_pattern, per-core list of per-expert
    actual sizes)."""
    E = len(splits)
    epc = E // N_CORES
    per_core = [tuple(int(s) for s in splits[c * epc : (c + 1) * epc]) for c in range(N_CORES)]
    uniform = all(p == per_core[0] for p in per_core)
    if uniform:
        padded = tuple(128 * math.ceil(s / 128) for s in per_core[0])
    else:
        m_pad = 128 * math.ceil(int(max(splits.max(), 1)) / 128)
        padded = (m_pad,) * epc
    return padded, per_core


def kernel(x: np.ndarray, W: np.ndarray, m_splits: np.ndarray, _profile=None) -> np.ndarray:
    x = np.ascontiguousarray(np.asarray(x), dtype=np.float32)
    W = np.ascontiguousarray(np.asarray(W), dtype=np.float32)
    splits = np.asarray(m_splits).astype(np.int64)
    E = splits.shape[0]
    assert E % N_CORES == 0 and W.shape[0] == E
    epc = E // N_CORES
    offs = np.concatenate([[0], np.cumsum(splits)])
    total = int(offs[-1])
    assert total == x.shape[0]

    padded, per_core = _plan(splits)
    pofs = np.concatenate([[0], np.cumsum(padded)])
    T_pad = int(pofs[-1])

    nc = _get_nc(padded)

    in_maps = []
    for c in range(N_CORES):
        if tuple(padded) == per_core[c]:
            xs = x[offs[c * epc] : offs[(c + 1) * epc]]
        else:
            xs = np.zeros((T_pad, IN_SIZE), dtype=np.float32)
            for e in range(epc):
                g = c * epc + e
                xs[pofs[e] : pofs[e] + splits[g]] = x[offs[g] : offs[g + 1]]
        xT = np.ascontiguousarray(xs.reshape(T_pad, KO, P).transpose(2, 1, 0))
        wT = np.ascontiguousarray(
            W[c * epc : (c + 1) * epc].reshape(epc, OUT_SIZE, KO, P).transpose(0, 3, 2, 1)
        )
        in_maps.append({"xT": xT, "wT": wT})

    kwargs = dict(_profile) if _profile else {}
    res = run_bass_kernel_spmd(nc, in_maps, core_ids=list(range(N_CORES)), **kwargs)
    if _profile is not None:
        _profile["result"] = res

    out = np.empty((total, OUT_SIZE), dtype=np.float32)
    for c in range(N_CORES):
        yc = res.results[c]["y"]
        for e in range(epc):
            g = c * epc + e
            out[offs[g] : offs[g + 1]] = yc[pofs[e] : pofs[e] + splits[g]]
    return out


# revision 4
# speedup vs baseline: 1.1276x; 1.1276x over previous
"""Trainium2 Bass kernel for MindSpeed TE grouped linear (MoE grouped GEMM).

Computes, for E=64 experts with row splits m_splits (sum = 32768):
    y[rows_e, :] = x[rows_e, :] @ W[e].T        W[e]: [1408, 2048]

Strategy: pure expert-parallel over 8 NeuronCores — core c owns experts
[8c, 8c+8) and their (contiguous) token rows. No collectives; gather is a
host-side concat. Host pre-transposes both operands into K-major layout
([P=128 partitions = contraction chunk, KO=16, free]) so the PE consumes
both sides directly, and the matmuls run as float32r (FP22 truncate,
single-pass, full PE rate; ~1.5e-4 rel err).
"""

import math

import numpy as np

import concourse.mybir as mybir
import concourse.tile as tile
from concourse import bacc
from concourse.bass_utils import run_bass_kernel_spmd

N_CORES = 8
P = 128
IN_SIZE = 2048
OUT_SIZE = 1408
KO = IN_SIZE // P  # 16 contraction subtiles
N_TILE = 512

_nc_cache: dict = {}


def _n_tiles():
    tiles = []
    n0 = 0
    while n0 < OUT_SIZE:
        nsz = min(N_TILE, OUT_SIZE - n0)
        tiles.append((n0, nsz))
        n0 += nsz
    return tiles


def _build(pattern: tuple) -> "bacc.Bacc":
    """One SPMD program: `pattern` = per-expert (padded) token counts for the
    8 local experts of a core; identical across cores."""
    T = sum(pattern)
    m_max = max(pattern)
    E_loc = len(pattern)
    nc = bacc.Bacc(None, target_bir_lowering=False, name="grouped_linear")
    xT = nc.dram_tensor("xT", [P, KO, T], mybir.dt.float32r, kind="ExternalInput")
    wT = nc.dram_tensor(
        "wT", [E_loc, P, KO, OUT_SIZE], mybir.dt.float32r, kind="ExternalInput"
    )
    y = nc.dram_tensor("y", [T, OUT_SIZE], mybir.dt.float32, kind="ExternalOutput")

    KH = KO // 2  # W arrives in two half-K chunks for finer pipelining
    with tile.TileContext(nc) as tc:
        with (
            tc.tile_pool(name="xp", bufs=8) as xpool,
            tc.tile_pool(name="wp", bufs=6) as wpool,
            tc.tile_pool(name="op", bufs=4) as opool,
            tc.tile_pool(name="ps", bufs=6, space="PSUM") as pspool,
        ):
            t0 = 0
            for e in range(E_loc):
                m = pattern[e]
                if m == 0:
                    continue
                mts = m // P
                x_ts = []
                for mt in range(mts):
                    x_t = xpool.tile([P, KO, P], mybir.dt.float32r, tag="x", name="x_t")
                    nc.sync.dma_start(
                        x_t[:], xT[:, :, t0 + mt * P : t0 + (mt + 1) * P]
                    )
                    x_ts.append(x_t)
                for n0, nsz in _n_tiles():
                    w_hs = []
                    for kh in range(2):
                        w_h = wpool.tile(
                            [P, KH, N_TILE], mybir.dt.float32r, tag="w", name="w_h"
                        )
                        nc.sync.dma_start(
                            w_h[:, :, :nsz],
                            wT[e, :, kh * KH : (kh + 1) * KH, n0 : n0 + nsz],
                        )
                        w_hs.append(w_h)
                    for mt in range(mts):
                        ps_t = pspool.tile(
                            [P, N_TILE], mybir.dt.float32, tag="ps", name="ps_t"
                        )
                        for kh in range(2):
                            for k in range(KH):
                                ko = kh * KH + k
                                nc.tensor.matmul(
                                    ps_t[:, :nsz],
                                    x_ts[mt][:, ko, :],
                                    w_hs[kh][:, k, :nsz],
                                    start=(ko == 0),
                                    stop=(ko == KO - 1),
                                )
                        o_t = opool.tile(
                            [P, N_TILE], mybir.dt.float32, tag="o", name="o_t"
                        )
                        nc.vector.tensor_copy(o_t[:, :nsz], ps_t[:, :nsz])
                        nc.scalar.dma_start(
                            y[t0 + mt * P : t0 + (mt + 1) * P, n0 : n0 + nsz],
                            o_t[:, :nsz],
                        )
                t0 += m
    nc.compile()
    return nc


def _get_nc(pattern: tuple) -> "bacc.Bacc":
    nc = _nc_cache.get(pattern)
    if nc is None:
        nc = _build(pattern)
        _nc_cache[pattern] = nc
    return nc


def _plan(splits: np.ndarray):
    """Choose a per-core expert-size pattern (identical across cores, sizes
    multiples of 128). Returns (padded_pattern, per-core list of per-expert
    actual sizes)."""
    E = len(splits)
    epc = E // N_CORES
    per_core = [tuple(int(s) for s in splits[c * epc : (c + 1) * epc]) for c in range(N_CORES)]
    uniform = all(p == per_core[0] for p in per_core)
    if uniform:
        padded = tuple(128 * math.ceil(s / 128) for s in per_core[0])
    else:
        m_pad = 128 * math.ceil(int(max(splits.max(), 1)) / 128)
        padded = (m_pad,) * epc
    return padded, per_core


def kernel(x: np.ndarray, W: np.ndarray, m_splits: np.ndarray, _profile=None) -> np.ndarray:
    x = np.ascontiguousarray(np.asarray(x), dtype=np.float32)
    W = np.ascontiguousarray(np.asarray(W), dtype=np.float32)
    splits = np.asarray(m_splits).astype(np.int64)
    E = splits.shape[0]
    assert E % N_CORES == 0 and W.shape[0] == E
    epc = E // N_CORES
    offs = np.concatenate([[0], np.cumsum(splits)])
    total = int(offs[-1])
    assert total == x.shape[0]

    padded, per_core = _plan(splits)
    pofs = np.concatenate([[0], np.cumsum(padded)])
    T_pad = int(pofs[-1])

    nc = _get_nc(padded)

    in_maps = []
    for c in range(N_CORES):
        if tuple(padded) == per_core[c]:
            xs = x[offs[c * epc] : offs[(c + 1) * epc]]
        else:
            xs = np.zeros((T_pad, IN_SIZE), dtype=np.float32)
            for e in range(epc):
                g = c * epc + e
                xs[pofs[e] : pofs[e] + splits[g]] = x[offs[g] : offs[g + 1]]
        xT = np.ascontiguousarray(xs.reshape(T_pad, KO, P).transpose(2, 1, 0))
        wT = np.ascontiguousarray(
            W[c * epc : (c + 1) * epc].reshape(epc, OUT_SIZE, KO, P).transpose(0, 3, 2, 1)
        )
        in_maps.append({"xT": xT, "wT": wT})

    kwargs = dict(_profile) if _profile else {}
    res = run_bass_kernel_spmd(nc, in_maps, core_ids=list(range(N_CORES)), **kwargs)
    if _profile is not None:
        _profile["result"] = res

    out = np.empty((total, OUT_SIZE), dtype=np.float32)
    for c in range(N_CORES):
        yc = res.results[c]["y"]
        for e in range(epc):
            g = c * epc + e
            out[offs[g] : offs[g + 1]] = yc[pofs[e] : pofs[e] + splits[g]]
    return out


# revision 5
# speedup vs baseline: 1.2839x; 1.1386x over previous
"""Trainium2 Bass kernel for MindSpeed TE grouped linear (MoE grouped GEMM).

Computes, for E=64 experts with row splits m_splits (sum = 32768):
    y[rows_e, :] = x[rows_e, :] @ W[e].T        W[e]: [1408, 2048]

Strategy: pure expert-parallel over 8 NeuronCores — core c owns experts
[8c, 8c+8) and their (contiguous) token rows. No collectives; gather is a
host-side concat. Host pre-transposes both operands into K-major layout
([P=128 partitions = contraction chunk, KO=16, free]) so the PE consumes
both sides directly, and the matmuls run as float32r (FP22 truncate,
single-pass, full PE rate; ~1.5e-4 rel err).
"""

import math

import numpy as np

import concourse.mybir as mybir
import concourse.tile as tile
from concourse import bacc
from concourse.bass_utils import run_bass_kernel_spmd

N_CORES = 8
P = 128
IN_SIZE = 2048
OUT_SIZE = 1408
KO = IN_SIZE // P  # 16 contraction subtiles
N_TILE = 512

_nc_cache: dict = {}


def _n_tiles():
    tiles = []
    n0 = 0
    while n0 < OUT_SIZE:
        nsz = min(N_TILE, OUT_SIZE - n0)
        tiles.append((n0, nsz))
        n0 += nsz
    return tiles


def _build(pattern: tuple) -> "bacc.Bacc":
    """One SPMD program: `pattern` = per-expert (padded) token counts for the
    8 local experts of a core; identical across cores."""
    T = sum(pattern)
    m_max = max(pattern)
    E_loc = len(pattern)
    nc = bacc.Bacc(None, target_bir_lowering=False, name="grouped_linear")
    xT = nc.dram_tensor("xT", [P, KO, T], mybir.dt.float32r, kind="ExternalInput")
    wT = nc.dram_tensor(
        "wT", [E_loc, P, KO, OUT_SIZE], mybir.dt.float32r, kind="ExternalInput"
    )
    y = nc.dram_tensor("y", [T, OUT_SIZE], mybir.dt.float32, kind="ExternalOutput")

    KQ = 4  # W arrives in quarter-K chunks (1 MB) for fine pipelining
    NQ = KO // KQ
    XC = 2 * P  # x granule: two m-tiles
    t_offs = np.concatenate([[0], np.cumsum(pattern)]).astype(int)
    # Process the larger expert of each consecutive pair first: the trailing
    # expert's compute is the pipeline drain, so ending small shrinks the tail.
    order = []
    for i in range(0, E_loc - 1, 2):
        a, b = i, i + 1
        order.extend([b, a] if pattern[b] > pattern[a] else [a, b])
    if E_loc % 2:
        order.append(E_loc - 1)

    with tile.TileContext(nc) as tc:
        with (
            tc.tile_pool(name="xp", bufs=5) as xpool,
            tc.tile_pool(name="wp", bufs=10) as wpool,
            tc.tile_pool(name="op", bufs=4) as opool,
            tc.tile_pool(name="ps", bufs=6, space="PSUM") as pspool,
        ):
            for e in order:
                m = pattern[e]
                if m == 0:
                    continue
                t0 = int(t_offs[e])
                mts = m // P
                x_cs = []
                for c0 in range(0, m, XC):
                    csz = min(XC, m - c0)
                    x_c = xpool.tile([P, KO, XC], mybir.dt.float32r, tag="x", name="x_c")
                    nc.sync.dma_start(
                        x_c[:, :, :csz], xT[:, :, t0 + c0 : t0 + c0 + csz]
                    )
                    x_cs.append(x_c)
                for n0, nsz in _n_tiles():
                    w_qs = []
                    for q in range(NQ):
                        w_q = wpool.tile(
                            [P, KQ, N_TILE], mybir.dt.float32r, tag="w", name="w_q"
                        )
                        nc.sync.dma_start(
                            w_q[:, :, :nsz],
                            wT[e, :, q * KQ : (q + 1) * KQ, n0 : n0 + nsz],
                        )
                        w_qs.append(w_q)
                    for mt in range(mts):
                        x_c = x_cs[mt // 2]
                        xoff = (mt % 2) * P
                        ps_t = pspool.tile(
                            [P, N_TILE], mybir.dt.float32, tag="ps", name="ps_t"
                        )
                        for q in range(NQ):
                            for k in range(KQ):
                                ko = q * KQ + k
                                nc.tensor.matmul(
                                    ps_t[:, :nsz],
                                    x_c[:, ko, xoff : xoff + P],
                                    w_qs[q][:, k, :nsz],
                                    start=(ko == 0),
                                    stop=(ko == KO - 1),
                                )
                        o_t = opool.tile(
                            [P, N_TILE], mybir.dt.float32, tag="o", name="o_t"
                        )
                        nc.vector.tensor_copy(o_t[:, :nsz], ps_t[:, :nsz])
                        nc.scalar.dma_start(
                            y[t0 + mt * P : t0 + (mt + 1) * P, n0 : n0 + nsz],
                            o_t[:, :nsz],
                        )
    nc.compile()
    return nc


def _get_nc(pattern: tuple) -> "bacc.Bacc":
    nc = _nc_cache.get(pattern)
    if nc is None:
        nc = _build(pattern)
        _nc_cache[pattern] = nc
    return nc


def _plan(splits: np.ndarray):
    """Choose a per-core expert-size pattern (identical across cores, sizes
    multiples of 128). Returns (padded_pattern, per-core list of per-expert
    actual sizes)."""
    E = len(splits)
    epc = E // N_CORES
    per_core = [tuple(int(s) for s in splits[c * epc : (c + 1) * epc]) for c in range(N_CORES)]
    uniform = all(p == per_core[0] for p in per_core)
    if uniform:
        padded = tuple(128 * math.ceil(s / 128) for s in per_core[0])
    else:
        m_pad = 128 * math.ceil(int(max(splits.max(), 1)) / 128)
        padded = (m_pad,) * epc
    return padded, per_core


def kernel(x: np.ndarray, W: np.ndarray, m_splits: np.ndarray, _profile=None) -> np.ndarray:
    x = np.ascontiguousarray(np.asarray(x), dtype=np.float32)
    W = np.ascontiguousarray(np.asarray(W), dtype=np.float32)
    splits = np.asarray(m_splits).astype(np.int64)
    E = splits.shape[0]
    assert E % N_CORES == 0 and W.shape[0] == E
    epc = E // N_CORES
    offs = np.concatenate([[0], np.cumsum(splits)])
    total = int(offs[-1])
    assert total == x.shape[0]

    padded, per_core = _plan(splits)
    pofs = np.concatenate([[0], np.cumsum(padded)])
    T_pad = int(pofs[-1])

    nc = _get_nc(padded)

    in_maps = []
    for c in range(N_CORES):
        if tuple(padded) == per_core[c]:
            xs = x[offs[c * epc] : offs[(c + 1) * epc]]
        else:
            xs = np.zeros((T_pad, IN_SIZE), dtype=np.float32)
            for e in range(epc):
                g = c * epc + e
                xs[pofs[e] : pofs[e] + splits[g]] = x[offs[g] : offs[g + 1]]
        xT = np.ascontiguousarray(xs.reshape(T_pad, KO, P).transpose(2, 1, 0))
        wT = np.ascontiguousarray(
            W[c * epc : (c + 1) * epc].reshape(epc, OUT_SIZE, KO, P).transpose(0, 3, 2, 1)
        )
        in_maps.append({"xT": xT, "wT": wT})

    kwargs = dict(_profile) if _profile else {}
    res = run_bass_kernel_spmd(nc, in_maps, core_ids=list(range(N_CORES)), **kwargs)
    if _profile is not None:
        _profile["result"] = res

    out = np.empty((total, OUT_SIZE), dtype=np.float32)
    for c in range(N_CORES):
        yc = res.results[c]["y"]
        for e in range(epc):
            g = c * epc + e
            out[offs[g] : offs[g + 1]] = yc[pofs[e] : pofs[e] + splits[g]]
    return out
